# revision 31
# baseline (speedup 1.0000x reference)
"""MixedDecoder (moe_routing) Trainium2 Bass kernel, v2.

Data-parallel over batch: B=1024 split as 128 samples per core across 8
NeuronCores.  All-bf16 mixed-expert datapath (weights, scaled inputs,
activations); fp32 gate MLP + softmax.

Per layer:  out = sum_e coeff[:,e] * (inp @ w[e]) + coeff @ b
PE-only accumulation: coeff scaling applied to K-transposed input tiles
(batched bf16 DVE multiply per expert), every (expert, k-tile) matmul plus
the mixed-bias matmul accumulates into one PSUM bank.  Ragged 64-row
k-tiles of two adjacent experts are merged into single [128,512] matmuls.

Weights are host-packed into SBUF-tile layout and streamed as one
contiguous DMA per (layer, expert) on the sync queue, in layer order, so
compute overlaps the stream and per-partition runs are 2-4KB.
"""

import numpy as np
import sys

sys.path.insert(0, "/opt/trn_rl_repo")

import concourse.bass as bass
import concourse.mybir as mybir
import concourse.tile as tile
from concourse.masks import make_identity
from concourse import library_config

F32 = mybir.dt.float32
BF16 = mybir.dt.bfloat16
AF = mybir.ActivationFunctionType
ALU = mybir.AluOpType

B, LAT, FCON = 1024, 64, 256
IN_SZ = LAT + FCON              # 320
HID, E, GATE_H = 512, 8, 64
INTER = LAT + HID               # 576
OUT_SZ = 512
NCORES = 8
BL = B // NCORES                # 128

LAST_EXEC_NS = None
LAST_RESULTS = None


def _split_multi_waits(bir_str):
    """Walrus accepts at most one sync wait per instruction; hoist extra
    on_wait entries onto standalone EventSemaphore instructions."""
    import json

    d = json.loads(bir_str)
    ctr = [0]

    def fix_list(lst):
        out = []
        for ins in lst:
            if isinstance(ins, dict) and "opcode" in ins and "sync_info" in ins:
                si = ins.get("sync_info") or {}
                ow = si.get("on_wait") or []
                if len(ow) > 1:
                    for w in ow[:-1]:
                        ctr[0] += 1
                        out.append({
                            "debug": ins.get("debug", 0),
                            "engine": ins["engine"],
                            "ins": [], "outs": [],
                            "name": f"splitwait_{ctr[0]}",
                            "opcode": "EventSemaphore",
                            "sync_info": {"on_update": [], "on_wait": [w]},
                        })
                    si["on_wait"] = [ow[-1]]
            out.append(ins)
        return out

    def walk(o):
        if isinstance(o, dict):
            for k, v in o.items():
                if (isinstance(v, list) and v and isinstance(v[0], dict)
                        and "opcode" in v[0]):
                    o[k] = fix_list(v)
                    for ins in o[k]:
                        walk(ins)
                else:
                    walk(v)
        elif isinstance(o, list):
            for v in o:
                walk(v)

    walk(d)
    return json.dumps(d).encode(), ctr[0]


def _install_wait_splitter():
    from concourse import bass2jax, bass_utils

    orig = bass_utils.compile_bir_kernel

    def wrapper(bir_str, *a, **k):
        if isinstance(bir_str, str):
            bir_str = bir_str.encode()
        new, n = _split_multi_waits(bir_str)
        return orig(new, *a, **k)

    bass2jax.compile_bir_kernel = wrapper
    return orig


def _elu(nc, pool, psum_ap, out_tile, P, N):
    """out = elu(psum):  relu(x) - relu(1 - exp(x)).  out may be bf16."""
    e = pool.tile([P, N], BF16, tag="elu_e")
    r = pool.tile([P, N], BF16, tag="elu_r")
    s = pool.tile([P, N], BF16, tag="elu_s")
    nc.scalar.activation(e[:], psum_ap, AF.Exp)
    nc.scalar.activation(r[:], psum_ap, AF.Relu)
    nc.scalar.activation(s[:], e[:], AF.Relu, bias=1.0, scale=-1.0)
    nc.vector.tensor_tensor(out=out_tile, in0=r[:], in1=s[:], op=ALU.subtract)


def build_program():
    nc = bass.Bass()

    # bf16 transposed gate weights: gwa = [g1w t0 | g1w t1] on 128 partitions,
    # gwb = [g1w t2 | g2w | g3w] on 64 partitions, gbt = fp32 biases [64, 3].
    gwa_d = nc.declare_dram_parameter("gwa", [128, 2 * GATE_H], BF16, isOutput=False)
    gwb_d = nc.declare_dram_parameter("gwb", [GATE_H, 2 * GATE_H + E], BF16, isOutput=False)
    gbt_d = nc.declare_dram_parameter("gbt", [GATE_H, 3], F32, isOutput=False)
    # bf16 mixed-layer inputs
    xb_d = nc.declare_dram_parameter("xb", [128, 512], BF16, isOutput=False)
    w0_d = nc.declare_dram_parameter("w0f", [128, E * 1024], BF16, isOutput=False)
    w1_d = nc.declare_dram_parameter("w1f", [128, E * 2048], BF16, isOutput=False)
    w2_d = nc.declare_dram_parameter("w2f", [128, E * 2048], BF16, isOutput=False)
    wz_d = nc.declare_dram_parameter("wz", [128, 12 * 512], BF16, isOutput=False)
    bias_d = nc.declare_dram_parameter("biasb", [E, 3 * 512], BF16, isOutput=False)
    oh_d = nc.declare_dram_parameter("ohb", [E, 12 * 128], BF16, isOutput=False)
    out_d = nc.declare_dram_parameter("out", [BL, OUT_SZ], F32, isOutput=True)

    with tile.TileContext(nc) as tc:
        with (
            tc.tile_pool(name="const", bufs=1) as cpool,
            tc.tile_pool(name="gate", bufs=1) as gpool,
            tc.tile_pool(name="acts", bufs=1) as apool,
            tc.tile_pool(name="elu", bufs=2) as epool,
            tc.tile_pool(name="wts", bufs=1) as wpool,
            tc.tile_pool(name="scaled", bufs=10) as spool,
            tc.tile_pool(name="zscaled", bufs=12) as zpool,
            tc.tile_pool(name="ps_main", bufs=2, space="PSUM") as ps_main,
            tc.tile_pool(name="ps_aux", bufs=2, space="PSUM") as ps_aux,
            tc.tile_pool(name="ps_tr", bufs=1, space="PSUM") as ps_tr_pool,
            tc.tile_pool(name="ps_bc", bufs=2, space="PSUM") as ps_bc,
        ):
            # ---- constants; preload the scalar activation table off the
            # critical path (first ACTIVATE triggers a ~1.3us table load)
            ident = cpool.tile([128, 128], F32)
            make_identity(nc, ident[:])
            identb = cpool.tile([128, 128], BF16, tag="identb")
            make_identity(nc, identb[:])
            dummy = cpool.tile([1, 1], F32, tag="dummy")
            nc.scalar.activation(dummy[:], ident[0:1, 0:1], AF.Exp)

            # PE warm-up: harmless matmuls while the gate DMAs land, so the
            # tensor engine p-state is ramped before real work arrives.
            ps_warm = ps_tr_pool.tile([128, 512], F32, tag="warm")
            for _ in range(30):
                nc.tensor.matmul(ps_warm[:, 0:128], identb[:], identb[:],
                                 start=True, stop=True)

            # gate-critical small DMAs at the head of the sync queue so
            # their packets reach the DMA engines before the weight stream
            xb = cpool.tile([128, 512], BF16, tag="xb")
            nc.sync.dma_start(xb[:], xb_d[:])
            gwa = gpool.tile([128, 2 * GATE_H], BF16, tag="gwa")
            nc.sync.dma_start(gwa[:], gwa_d[:])
            gwb = gpool.tile([GATE_H, 2 * GATE_H + E], BF16, tag="gwb")
            nc.sync.dma_start(gwb[:], gwb_d[:])
            gbt = gpool.tile([GATE_H, 3], F32, tag="gbt")
            nc.sync.dma_start(gbt[:], gbt_d[:])
            ohb = cpool.tile([E, 12 * 128], BF16, tag="ohb")
            nc.sync.dma_start(ohb[:], oh_d[:])
            # non-urgent small DMAs on scalar queue
            biasb = cpool.tile([E, 3 * 512], BF16, tag="biasb")
            nc.scalar.dma_start(biasb[:], bias_d[:])

            # ---- weight streaming on sync queue, in consumption order
            w0t, w1t, w2t, wzt = [], [], [], []
            t = wpool.tile([128, 2048], BF16, tag="wz0")
            nc.sync.dma_start(t[:], wz_d[:, 0:2048])
            wzt.append(t)
            for e in range(E):
                t = wpool.tile([128, 1024], BF16, tag=f"w0e{e}")
                nc.sync.dma_start(t[:], w0_d[:, e * 1024 : (e + 1) * 1024])
                w0t.append(t)
            t = wpool.tile([128, 2048], BF16, tag="wz1")
            nc.sync.dma_start(t[:], wz_d[:, 2048:4096])
            wzt.append(t)
            for e in range(E):
                t = wpool.tile([128, 2048], BF16, tag=f"w1e{e}")
                nc.sync.dma_start(t[:], w1_d[:, e * 2048 : (e + 1) * 2048])
                w1t.append(t)
            t = wpool.tile([128, 2048], BF16, tag="wz2")
            nc.sync.dma_start(t[:], wz_d[:, 4096:6144])
            wzt.append(t)
            for e in range(E):
                t = wpool.tile([128, 2048], BF16, tag=f"w2e{e}")
                nc.sync.dma_start(t[:], w2_d[:, e * 2048 : (e + 1) * 2048])
                w2t.append(t)

            def warm(n, rhs):
                for _ in range(n):
                    nc.tensor.matmul(ps_warm[:], identb[:],
                                     rhs, start=True, stop=True)

            # ---- gate MLP, transposed bf16: h^T [64, 128] per layer, biases
            # folded into the scalar-engine activations (per-partition).
            def elu_t(ps_ap, out_tile, bias_ap):
                eg = epool.tile([GATE_H, BL], F32, tag="ge")
                rg = epool.tile([GATE_H, BL], F32, tag="gr")
                sg = epool.tile([GATE_H, BL], F32, tag="gs")
                nc.scalar.activation(eg[:], ps_ap, AF.Exp, bias=bias_ap)
                nc.vector.tensor_scalar(out=rg[:], in0=ps_ap, scalar1=bias_ap,
                                        scalar2=0.0, op0=ALU.add, op1=ALU.max)
                nc.scalar.activation(sg[:], eg[:], AF.Relu, bias=1.0, scale=-1.0)
                nc.vector.tensor_tensor(out=out_tile, in0=rg[:], in1=sg[:],
                                        op=ALU.subtract)

            ps_g = ps_aux.tile([GATE_H, BL], F32, tag="g")
            nc.tensor.matmul(ps_g[:], gwa[:, 0:GATE_H], xb[:, 0:128],
                             start=True, stop=False)
            nc.tensor.matmul(ps_g[:], gwa[:, GATE_H : 2 * GATE_H],
                             xb[:, 128:256], start=False, stop=False)
            nc.tensor.matmul(ps_g[:], gwb[:, 0:GATE_H], xb[0:GATE_H, 256:384],
                             start=False, stop=True)
            h1t = gpool.tile([GATE_H, BL], BF16, tag="h1t")
            elu_t(ps_g[:], h1t[:], gbt[:, 0:1])

            ps_g2 = ps_aux.tile([GATE_H, BL], F32, tag="g")
            nc.tensor.matmul(ps_g2[:], gwb[:, GATE_H : 2 * GATE_H], h1t[:],
                             start=True, stop=True)
            h2t = gpool.tile([GATE_H, BL], BF16, tag="h2t")
            elu_t(ps_g2[:], h2t[:], gbt[:, 1:2])

            ps_g3 = ps_aux.tile([E, BL], F32, tag="g")
            nc.tensor.matmul(ps_g3[:], gwb[:, 2 * GATE_H : 2 * GATE_H + E],
                             h2t[:], start=True, stop=True)
            # UNNORMALIZED softmax numerators (no max-subtraction: |logits|
            # is small).  The 1/sum(exp) normalization is folded into each
            # mixed layer's elu as a per-partition (per-sample) scale, so
            # the gate critical path ends right here at ct.
            for _ in range(8):
                nc.tensor.matmul(ps_warm[:, 0:128], identb[:], identb[:],
                                 start=True, stop=True)
            enumt = gpool.tile([E, BL], F32, tag="enumt")
            nc.scalar.activation(enumt[:], ps_g3[:], AF.Exp,
                                 bias=gbt[0:E, 2:3])
            ct4b = gpool.tile([E, 128], BF16, tag="ct4b")
            nc.vector.tensor_copy(ct4b[:], enumt[:])

            # broadcast tiles: bcs[g][p, (j,b)] = coeff[b, 4g+j]; bcs[2] = pairs
            # (pairs first: they feed the z-pair scalings and first matmuls)
            bcs = [None, None, None]
            for g in (2, 0, 1):
                ps_b = ps_bc.tile([128, 512], F32, tag="bc")
                for j in range(4):
                    nc.tensor.matmul(
                        ps_b[:, j * 128 : (j + 1) * 128],
                        ohb[:, (g * 4 + j) * 128 : (g * 4 + j + 1) * 128],
                        ct4b[:],
                        start=True, stop=True,
                    )
                sb = apool.tile([128, 512], BF16, tag=f"bc{g}")
                if g == 2:
                    nc.vector.tensor_copy(sb[:], ps_b[:])
                else:
                    nc.scalar.activation(sb[:], ps_b[:], AF.Copy)
                bcs[g] = sb

            def bc_e(e):
                return bcs[e // 4][:, (e % 4) * 128 : (e % 4 + 1) * 128]

            def bc_pair(i):
                return bcs[2][:, i * 128 : (i + 1) * 128]

            # pre-scale all merged z-pair tiles (DVE, don't depend on h)
            zscaled = {}
            for li, src0 in [(0, 256), (1, 384), (2, 384)]:
                for i in range(4):
                    az = zpool.tile([128, 128], BF16, tag="az")
                    nc.vector.tensor_tensor(
                        out=az[:], in0=xb[:, src0 : src0 + 128],
                        in1=bc_pair(i), op=ALU.mult,
                    )
                    zscaled[(li, i)] = az

            # softmax denominator -> rec [128,1]; off the critical path
            ps_en = ps_aux.tile([128, E], F32, tag="g")
            nc.tensor.transpose(ps_en[:, 0:E], enumt[:], ident[0:E, 0:E])
            esum = gpool.tile([128, 1], F32)
            nc.vector.tensor_reduce(esum[:], ps_en[:, 0:E],
                                    axis=mybir.AxisListType.X, op=ALU.add)
            rec = gpool.tile([128, 1], F32)
            nc.vector.reciprocal(rec[:], esum[:])

            # ---- 3 mixed-expert layers.  Per layer: bias + z-pair matmuls
            # first (they only need ct), then the previous layer's h
            # transposes (PE filler ordering keeps the PE busy across the
            # elu chain), then the full k-tile matmuls.
            layer_cfg = [
                (w0t, 2, 0, HID, True),
                (w1t, 4, 1, HID, True),
                (w2t, 4, 2, OUT_SZ, False),
            ]
            hc = None  # previous layer's h chunks (4 x [128,128] bf16)
            for li, (wt, ntile, lz, NOUT, has_act) in enumerate(layer_cfg):
                ps_o = ps_main.tile([128, NOUT], F32)
                nc.tensor.matmul(
                    ps_o[:], ct4b[:],
                    biasb[:, li * 512 : (li + 1) * 512],
                    start=True, stop=False,
                )
                for i in range(4):
                    nc.tensor.matmul(
                        ps_o[:], zscaled[(li, i)][:],
                        wzt[lz][:, i * 512 : (i + 1) * 512],
                        start=False, stop=False,
                    )
                if li > 0:
                    warm(4, wt[2][:, 0:512])
                # transpose previous layer's h chunks -> hT (after the
                # bias/z filler so the PE has work while elu completes);
                # expert 0's scaled tiles are chunked so its matmuls can
                # start as soon as each hT chunk lands.
                if li > 0:
                    ps_tr = ps_tr_pool.tile([128, 512], BF16, tag="ps_tr_h")
                    hT = apool.tile([128, 512], BF16, tag=f"hT{li}")
                    a0c, a1c = [], []
                    for t in range(4):
                        sl = slice(t * 128, (t + 1) * 128)
                        nc.tensor.transpose(ps_tr[:, sl], hc[t][:], identb[:])
                        nc.vector.tensor_copy(hT[:, sl], ps_tr[:, sl])
                        for ci, lst in ((0, a0c), (1, a1c)):
                            ac = spool.tile([128, 128], BF16, tag="a0c")
                            nc.vector.tensor_tensor(
                                out=ac[:], in0=hT[:, sl], in1=bc_e(ci),
                                op=ALU.mult,
                            )
                            lst.append(ac)
                src = xb[:, 0:256] if li == 0 else hT[:]
                for e in range(E):
                    if li > 0 and e < 2:
                        for t in range(ntile):
                            nc.tensor.matmul(
                                ps_o[:], (a0c if e == 0 else a1c)[t][:],
                                wt[e][:, t * 512 : (t + 1) * 512],
                                start=False, stop=False,
                            )
                        continue
                    a = spool.tile([128, 512], BF16, tag="a")
                    nc.vector.tensor_tensor(
                        out=a[:, 0 : ntile * 128].rearrange(
                            "p (t b) -> p t b", t=ntile),
                        in0=src.rearrange("p (t b) -> p t b", t=ntile),
                        in1=bc_e(e).unsqueeze(1).broadcast_to([128, ntile, 128]),
                        op=ALU.mult,
                    )
                    for t in range(ntile):
                        nc.tensor.matmul(
                            ps_o[:], a[:, t * 128 : (t + 1) * 128],
                            wt[e][:, t * 512 : (t + 1) * 512],
                            start=False, stop=(e == E - 1 and t == ntile - 1),
                        )

                if has_act:
                    # elu with the softmax normalization folded in as a
                    # per-partition scale; full-width scalar activations
                    # (scalar ops have ~400ns fixed cost), chunked DVE
                    # subtracts so each transpose waits only on its chunk
                    e_ = epool.tile([128, NOUT], BF16, tag="elu_e")
                    r_ = epool.tile([128, NOUT], BF16, tag="elu_r")
                    s_ = epool.tile([128, NOUT], BF16, tag="elu_s")
                    nc.scalar.activation(e_[:], ps_o[:], AF.Exp, scale=rec[:])
                    nc.vector.tensor_scalar(out=r_[:], in0=ps_o[:],
                                            scalar1=rec[:], scalar2=0.0,
                                            op0=ALU.mult, op1=ALU.max)
                    nc.scalar.activation(s_[:], e_[:], AF.Relu,
                                         bias=1.0, scale=-1.0)
                    hc = []
                    for t in range(4):
                        sl = slice(t * 128, (t + 1) * 128)
                        ht = apool.tile([128, 128], BF16, tag=f"h{li}_{t}")
                        nc.vector.tensor_tensor(out=ht[:], in0=r_[:, sl],
                                                in1=s_[:, sl], op=ALU.subtract)
                        hc.append(ht)
                else:
                    res = apool.tile([128, NOUT], F32, tag="res")
                    for j in range(2):
                        sl = slice(j * 256, (j + 1) * 256)
                        nc.vector.tensor_scalar_mul(res[:, sl], ps_o[:, sl],
                                                    rec[:])
                        nc.sync.dma_start(out_d[:, sl], res[:, sl])

    return nc


def prepare_in_maps(z, c, w0, b0, w1, b1, w2, b2,
                    gw1, gb1, gw2, gb2, gw3, gb3):
    import ml_dtypes
    bf = ml_dtypes.bfloat16
    f = np.float32

    x = np.concatenate([z, c], axis=1).astype(f)                  # [B, 320]
    gwa = np.concatenate([gw1[0:128, :], gw1[128:256, :]], axis=1).astype(bf)
    gwb = np.concatenate([gw1[256:320, :], gw2, gw3], axis=1).astype(bf)
    gbt = np.zeros((GATE_H, 3), f)
    gbt[:, 0] = gb1
    gbt[:, 1] = gb2
    gbt[0:E, 2] = gb3

    def pack_full(w, r0, ntiles):
        # [128, E*ntiles*512]: col e*ntiles*512 + t*512 + o = w[e, r0+t*128+p, o]
        return np.ascontiguousarray(
            w[:, r0 : r0 + ntiles * 128, :]
            .reshape(E, ntiles, 128, 512)
            .transpose(2, 0, 1, 3)
            .reshape(128, E * ntiles * 512)
            .astype(bf)
        )

    def pack_z(w, r0):
        # [128, 4*512]: block i: p<64 -> w[2i, r0+p, :]; p>=64 -> w[2i+1, ...]
        blocks = []
        for i in range(4):
            blocks.append(
                np.concatenate([w[2 * i, r0 : r0 + 64, :],
                                w[2 * i + 1, r0 : r0 + 64, :]], axis=0)
            )
        return np.concatenate(blocks, axis=1).astype(bf)  # [128, 2048]

    w0f = pack_full(w0, 0, 2)
    w1f = pack_full(w1, 64, 4)
    w2f = pack_full(w2, 64, 4)
    wz = np.concatenate(
        [pack_z(w0, 256), pack_z(w1, 0), pack_z(w2, 0)], axis=1
    )  # [128, 6144]
    biasb = np.concatenate([b0, b1, b2], axis=1).astype(bf)       # [8, 1536]

    ohb = np.zeros((E, 12 * 128), np.float32)
    for e in range(E):
        ohb[e, e * 128 : (e + 1) * 128] = 1.0
    for i in range(4):
        ohb[2 * i, (8 + i) * 128 : (8 + i) * 128 + 64] = 1.0
        ohb[2 * i + 1, (8 + i) * 128 + 64 : (9 + i) * 128] = 1.0
    ohb = ohb.astype(bf)

    shared = {
        "gwa": gwa, "gwb": gwb, "gbt": gbt,
        "w0f": w0f, "w1f": w1f, "w2f": w2f, "wz": wz,
        "biasb": biasb, "ohb": ohb,
    }
    in_maps = []
    for i in range(NCORES):
        xT = np.ascontiguousarray(x.T[:, i * BL : (i + 1) * BL])  # [320, BL]
        xbb = np.concatenate([
            xT[0:128, :],
            xT[128:256, :],
            np.concatenate([xT[256:320, :], xT[256:320, :]], axis=0),
            np.concatenate([xT[0:64, :], xT[0:64, :]], axis=0),
        ], axis=1).astype(bf)                                      # [128, 512]
        m = dict(shared)
        m["xb"] = xbb
        in_maps.append(m)
    return in_maps


def kernel(z, c, w0, b0, w1, b1, w2, b2, gw1, gb1, gw2, gb2, gw3, gb3):
    global LAST_EXEC_NS, LAST_RESULTS
    from concourse.bass_utils import run_bass_kernel_spmd

    _install_wait_splitter()
    in_maps = prepare_in_maps(z, c, w0, b0, w1, b1, w2, b2,
                              gw1, gb1, gw2, gb2, gw3, gb3)
    nc = build_program()
    r = run_bass_kernel_spmd(nc, in_maps, list(range(NCORES)))
    LAST_EXEC_NS = r.exec_time_ns
    LAST_RESULTS = r
    return np.concatenate([r.results[i]["out"] for i in range(NCORES)], axis=0)
